# revision 14
# baseline (speedup 1.0000x reference)
"""Trainium2 Bass kernel for nn_BatchTreeEncoder (batched tree-GRU encoder).

Strategy
--------
Pure data parallel over the batch: 256 trees -> 32 trees on each of the 8
NeuronCores, weights replicated.  Activations are kept in a transposed
[E, nodes] layout (E-chunks of 128 on partitions, nodes on the free dim).

Key ideas:
  - GX[v] = 64*(Wi @ emb[v] + bias_fold) is precomputed on the HOST into a
    [VOCAB, 3E] bf16 table; the kernel indirect-DMA gathers GX rows and
    injects them transposed into the gate PSUM banks via identity matmuls
    (start=True), onto which the recurrent matmuls (Wh fp8 x8 DoubleRow on
    h fp8 x8 -> PSUM x64) accumulate (start=False).
  - r/z share one [128, 2, n] PSUM tile and a single merged activation.
  - software pipelining: chunk i's GRU (S1) is emitted before chunk i-1's
    attention (S2); small levels are split in two chunks so the pipeline
    crosses level boundaries (parents of chunk a only need children of
    chunk a).
  - attention softmax normalization runs 128-partition-wide after an
    all-partition broadcast of the exp row (matmul with ones), avoiding
    single-lane row ops; the weighted child sum runs on the Pool engine.
  - running per-tree elementwise max folded in as each H chunk completes.
Output: PE-transpose of the [E, 32] max back to [32, E] and DMA out.
"""

import sys

for _p in ("/opt/trn_rl_repo",):
    if _p not in sys.path:
        sys.path.insert(0, _p)

import numpy as np
import ml_dtypes

bf16 = ml_dtypes.bfloat16
fp8t = ml_dtypes.float8_e4m3

# ---------------------------------------------------------------- constants
NCORES = 8
BS = 256
T = BS // NCORES          # trees per core
K = 3
DEPTH = 4
E = 1024
EC = E // 128             # 8 e-chunks
VOCAB = 20000
N_NODES = sum(K**l for l in range(DEPTH + 1))   # 121
LEVEL_OFF = [sum(K**i for i in range(l)) for l in range(DEPTH + 1)]  # [0,1,4,13,40]

S_W = 8.0                 # fp8 weight scale (Wh, Ws)
S_H = 8.0                 # fp8 hidden scale
S_GX = S_W * S_H          # 64: GX table scale == gh psum scale

# node-chunk sizes per level; multiples of 3^l (whole trees inside one chunk
# for the max) and of 3 for l>0 (whole sibling groups for the attention).
# Levels are split >=2 ways so S1(i+1) can overlap S2(i) across levels.
CHUNK_SIZES = {4: [486] * 5 + [162], 3: [432, 432], 2: [144, 144],
               1: [48, 48], 0: [16, 16]}


def _schedule():
    """Static per-core schedule: list of (level, c0, nc, [(gblock, boff, rows)])."""
    sched = []
    gb = 0
    for l in range(DEPTH, -1, -1):
        c0 = 0
        for nc_ in CHUNK_SIZES[l]:
            blocks = []
            boff = 0
            while boff < nc_:
                rows = min(128, nc_ - boff)
                blocks.append((gb, boff, rows))
                gb += 1
                boff += rows
            sched.append((l, c0, nc_, blocks))
            c0 += nc_
    return sched, gb


SCHEDULE, NB = _schedule()

_NC_CACHE = {}


# ---------------------------------------------------------------- builder
def build_nc():
    import concourse.bacc as bacc
    import concourse.bass as bass
    import concourse.mybir as mybir
    import concourse.tile as tile
    from concourse.masks import make_identity

    dt = mybir.dt
    Act = mybir.ActivationFunctionType
    Alu = mybir.AluOpType
    X = mybir.AxisListType.X

    nc = bacc.Bacc("TRN2", target_bir_lowering=False, debug=False)

    tok_d = nc.dram_tensor("tok", [NB, 128], dt.int32, kind="ExternalInput")
    gx_d = nc.dram_tensor("gx", [VOCAB, 3 * E], dt.bfloat16, kind="ExternalInput")
    whT_d = nc.dram_tensor("whT", [3 * EC, EC // 2, 128, 2, 128], dt.float8e4, kind="ExternalInput")
    ws_d = nc.dram_tensor("ws", [EC, EC // 2, 128, 2, 128], dt.float8e4, kind="ExternalInput")
    ctx_d = nc.dram_tensor("ctxw", [EC, 128, 1], dt.bfloat16, kind="ExternalInput")
    bias_d = nc.dram_tensor("bias", [128, 16], dt.float32, kind="ExternalInput")
    out_d = nc.dram_tensor("out", [T, E], dt.float32, kind="ExternalOutput")

    from contextlib import ExitStack

    DR = mybir.MatmulPerfMode.DoubleRow

    with tile.TileContext(nc) as tc, ExitStack() as ctx:
        sing = ctx.enter_context(tc.tile_pool(name="sing", bufs=1))
        hsp = ctx.enter_context(tc.tile_pool(name="hsp", bufs=1))
        mp_ = ctx.enter_context(tc.tile_pool(name="mp", bufs=1))
        gxp = ctx.enter_context(tc.tile_pool(name="gxp", bufs=7))
        gp = ctx.enter_context(tc.tile_pool(name="gp", bufs=2))      # gate tiles
        hp = ctx.enter_context(tc.tile_pool(name="hp", bufs=2))      # H chunks
        up = ctx.enter_context(tc.tile_pool(name="up", bufs=2))      # U tiles
        rowp = ctx.enter_context(tc.tile_pool(name="rowp", bufs=2))  # softmax rows
        wp = ctx.enter_context(tc.tile_pool(name="wp", bufs=2))      # bcast weights etc
        psp = ctx.enter_context(tc.tile_pool(name="psp", bufs=1, space="PSUM"))

        # ---- persistent / constant tiles
        whT = sing.tile([128, 3 * EC, EC // 2, 2, 128], dt.float8e4)
        ws = sing.tile([128, EC, EC // 2, 2, 128], dt.float8e4)
        ctxw = sing.tile([128, EC, 1], dt.bfloat16)
        biases = sing.tile([128, 16], dt.float32)
        identb = sing.tile([128, 128], dt.bfloat16)
        identf = sing.tile([128, 128], dt.float32)
        ones = sing.tile([1, 128], dt.bfloat16)
        idx = sing.tile([128, NB], dt.int32)

        nc.sync.dma_start(out=idx[:], in_=tok_d.rearrange("b p -> p b"))
        nc.sync.dma_start(out=biases[:], in_=bias_d[:])
        nc.sync.dma_start(out=ctxw[:, :, 0], in_=ctx_d.rearrange("k p o -> p (k o)"))
        make_identity(nc, identb[:])
        make_identity(nc, identf[:])
        nc.vector.memset(ones[:], 1.0)

        # bias column helpers: cols 0..7 = 64*bh_n, 8..15 = sent_bias
        def bcol(c):
            return biases[:, c:c + 1]

        # running max, [128, EC, T] f32
        msb = mp_.tile([128, EC, T], dt.float32)
        red = mp_.tile([128, EC, T], dt.float32)
        nc.vector.memset(msb[:], -3.0e38)

        # per-level HS accumulation targets ([E, N_l] as [128, EC, N_l])
        hs_sb = {}
        hs8_sb = {}
        for l in range(DEPTH):
            n_l = T * K**l
            hs_sb[l] = hsp.tile([128, EC, n_l], dt.bfloat16, name=f"hs{l}")
            hs8_sb[l] = hsp.tile([128, EC, n_l], dt.float8e4, name=f"hs8{l}")

        def gh_mms(out_ap, g, lvl, c0, ncn, start):
            src8 = hs8_sb[lvl]
            for j in range(EC // 2):
                nc.tensor.matmul(
                    out=out_ap, lhsT=whT[:, g, j, :, :],
                    rhs=src8[:, 2 * j:2 * j + 2, c0:c0 + ncn],
                    start=(start and j == 0), stop=(j == EC // 2 - 1),
                    perf_mode=DR, skip_group_check=not start)

        def u_mms(out_ap, f, h8, ncn):
            for j in range(EC // 2):
                nc.tensor.matmul(
                    out=out_ap, lhsT=ws[:, f, j, :, :],
                    rhs=h8[:, 2 * j:2 * j + 2, :ncn],
                    start=(j == 0), stop=(j == EC // 2 - 1), perf_mode=DR)

        # gather GX rows for one chunk; returns list of (tile, boff, rows)
        def emit_gather(blocks):
            rowsl = []
            for (gb, boff, rows) in blocks:
                gxrow = gxp.tile([128, 3 * E], dt.bfloat16, name="gxrow")
                nc.gpsimd.indirect_dma_start(
                    out=gxrow[:rows, :],
                    out_offset=None,
                    in_=gx_d[:, :],
                    in_offset=bass.IndirectOffsetOnAxis(ap=idx[:rows, gb:gb + 1], axis=0),
                )
                rowsl.append((gxrow, boff, rows))
            return rowsl

        # inject gathered gx gate-columns transposed into a psum region via a
        # regular matmul (out = gxrow_slice^T @ I); start=True resets the
        # region so the gh matmuls can accumulate on top with start=False
        def gx_tr(ps_ap_base, gxrows, gate, e):
            col0 = gate * E + e * 128
            for (gxrow, boff, rows) in gxrows:
                nc.tensor.matmul(
                    out=ps_ap_base[:, boff:boff + rows],
                    lhsT=gxrow[:rows, col0:col0 + 128],
                    rhs=identb[:rows, :rows],
                    start=True, stop=True,
                )

        # ---------------- S1: gather -> GRU gates -> H, hch8, running max
        def emit_s1(ci, gxrows):
            lvl, c0, ncn, blocks = SCHEDULE[ci]
            leaf = lvl == DEPTH
            n_per_tree = K**lvl
            tr0 = c0 // n_per_tree
            ntr = ncn // n_per_tree

            rz = gp.tile([128, 2, EC, 512], dt.bfloat16, name="rz", tag="rz")
            nt = gp.tile([128, EC, 512], dt.bfloat16, name="nt", tag="nt")
            hch = hp.tile([128, EC, 512], dt.bfloat16, name="hch")
            hch8 = None
            if lvl > 0:
                hch8 = hp.tile([128, EC, 512], dt.float8e4, name="hch8", tag="hch8")

            # r+z gates share one 2-bank psum and one merged activation
            for e in range(EC):
                psrz = psp.tile([128, 2, 512], dt.float32, name="psrz", tag="acc", bufs=2)
                gx_tr(psrz[:, 0, :], gxrows, 0, e)
                gx_tr(psrz[:, 1, :], gxrows, 1, e)
                if not leaf:
                    gh_mms(psrz[:, 0, :ncn], e, lvl, c0, ncn, start=False)
                    gh_mms(psrz[:, 1, :ncn], EC + e, lvl, c0, ncn, start=False)
                nc.scalar.activation(rz[:, :, e, :ncn], psrz[:, :, :ncn], Act.Tanh,
                                     scale=0.5 / S_GX)
            # r = 0.5 + 0.5*tau (batched over e)
            nc.vector.tensor_scalar(rz[:, 0, :, :ncn], rz[:, 0, :, :ncn], 0.5, 0.5,
                                    Alu.mult, Alu.add)
            # n gate
            for e in range(EC):
                psx = psp.tile([128, 512], dt.float32, name="psx", tag="gxn", bufs=2)
                gx_tr(psx, gxrows, 2, e)
                tt = gp.tile([128, 512], dt.float32, name="tt", tag="tt")
                if leaf:
                    # tt = r * 64bh_n + GXn64
                    nc.vector.scalar_tensor_tensor(
                        out=tt[:, :ncn], in0=rz[:, 0, e, :ncn], scalar=bcol(e),
                        in1=psx[:, :ncn], op0=Alu.mult, op1=Alu.add)
                else:
                    psh = psp.tile([128, 512], dt.float32, name="psh", tag="ghn", bufs=2)
                    gh_mms(psh[:, :ncn], 2 * EC + e, lvl, c0, ncn, start=True)
                    # tt = (GHn64 + 64bh_n) * r ; then += GXn64
                    nc.vector.scalar_tensor_tensor(
                        out=tt[:, :ncn], in0=psh[:, :ncn], scalar=bcol(e),
                        in1=rz[:, 0, e, :ncn], op0=Alu.add, op1=Alu.mult)
                    nc.vector.tensor_add(tt[:, :ncn], tt[:, :ncn], psx[:, :ncn])
                nc.scalar.activation(nt[:, e, :ncn], tt[:, :ncn], Act.Tanh,
                                     scale=1.0 / S_GX)
            # ---------------- blend -> H; e-halves split across Vector/Pool
            # (Pool supports only single-scalar tensor_scalar forms)
            H = EC // 2
            for eng, sl in ((nc.vector, slice(0, H)), (nc.gpsimd, slice(H, EC))):
                zt = rz[:, 1, sl, :ncn]
                nts = nt[:, sl, :ncn]
                pool = eng is nc.gpsimd
                if leaf:
                    if pool:
                        # h = 0.5*(n - zeta*n)
                        eng.tensor_mul(zt, zt, nts)
                        dd = rz[:, 0, sl, :ncn]
                        eng.tensor_sub(dd, nts, zt)
                        eng.tensor_scalar_mul(hch[:, sl, :ncn], dd, 0.5)
                    else:
                        eng.tensor_scalar(zt, zt, -0.5, 0.5, Alu.mult, Alu.add)
                        eng.tensor_mul(hch[:, sl, :ncn], zt, nts)
                else:
                    dd = rz[:, 0, sl, :ncn]  # r is dead; reuse as blend scratch
                    eng.tensor_sub(dd, hs_sb[lvl][:, sl, c0:c0 + ncn], nts)
                    eng.tensor_mul(zt, zt, dd)
                    eng.tensor_add(dd, dd, zt)
                    if pool:
                        eng.tensor_scalar_mul(dd, dd, 0.5)
                        eng.tensor_add(hch[:, sl, :ncn], dd, nts)
                    else:
                        eng.scalar_tensor_tensor(
                            out=hch[:, sl, :ncn], in0=dd, scalar=0.5,
                            in1=nts, op0=Alu.mult, op1=Alu.add)
                if hch8 is not None:
                    if pool:
                        eng.tensor_scalar_mul(hch8[:, sl, :ncn],
                                              hch[:, sl, :ncn], S_H)
                    else:
                        eng.tensor_scalar(hch8[:, sl, :ncn], hch[:, sl, :ncn],
                                          S_H, 0.0, Alu.mult, Alu.add)
            # ---------------- running max (batched over e)
            if n_per_tree == 1:
                nc.vector.tensor_max(msb[:, :, tr0:tr0 + ntr],
                                     msb[:, :, tr0:tr0 + ntr], hch[:, :, :ncn])
            else:
                nc.vector.reduce_max(
                    out=red[:, :, :ntr],
                    in_=hch[:, :, :ncn].rearrange("p e (t n) -> p e t n",
                                                  n=n_per_tree),
                    axis=X)
                nc.vector.tensor_max(msb[:, :, tr0:tr0 + ntr],
                                     msb[:, :, tr0:tr0 + ntr], red[:, :, :ntr])
            return hch, hch8

        # ---------------- S2: child attention -> hs, hs8 for parent level
        def emit_s2(ci, hch, hch8):
            lvl, c0, ncn, blocks = SCHEDULE[ci]
            npar = ncn // 3
            p0 = c0 // 3
            lp = lvl - 1
            pss_t = psp.tile([128, 512], dt.float32, name="pss", tag="gxn", bufs=2)
            pss = pss_t[0:1, :]
            for f in range(EC):
                psu_t = psp.tile([128, 2, 512], dt.float32, name="psu", tag="acc", bufs=2)
                psu = psu_t[:, 0, :]
                u_mms(psu[:, :ncn], f, hch8, ncn)
                ut = up.tile([128, 512], dt.bfloat16, name="ut", tag="ut")
                nc.scalar.activation(ut[:, :ncn], psu[:, :ncn], Act.Tanh,
                                     bias=bcol(8 + f), scale=1.0 / S_GX)
                nc.tensor.matmul(out=pss[:, :ncn], lhsT=ctxw[:, f, 0:1],
                                 rhs=ut[:, :ncn],
                                 start=(f == 0), stop=(f == EC - 1))
            srow = rowp.tile([1, 512], dt.float32, name="srow", tag="srow")
            nc.scalar.activation(srow[:, :ncn], pss[:, :ncn], Act.Tanh)
            erow = rowp.tile([1, 512], dt.bfloat16, name="erow", tag="erow")
            nc.scalar.activation(erow[:, :ncn], srow[:, :ncn], Act.Exp)
            # broadcast exp row to 128 partitions, then normalize wide into
            # prenormalized weights wb
            psw = psp.tile([128, 512], dt.float32, name="psw", tag="ghn", bufs=2)
            nc.tensor.matmul(out=psw[:, :ncn], lhsT=ones[:, :],
                             rhs=erow[:, :ncn], start=True, stop=True)
            eb = wp.tile([128, 512], dt.bfloat16, name="eb", tag="eb")
            nc.vector.tensor_copy(out=eb[:, :ncn], in_=psw[:, :ncn])
            eb3 = eb[:, :ncn].rearrange("p (n k) -> p n k", k=3)
            d3 = wp.tile([128, 176], dt.float32, name="d3", tag="d3")
            nc.vector.tensor_add(d3[:, :npar], eb3[:, :, 0], eb3[:, :, 1])
            nc.vector.tensor_add(d3[:, :npar], d3[:, :npar], eb3[:, :, 2])
            rinv = wp.tile([128, 176], dt.float32, name="rinv", tag="rinv")
            nc.vector.reciprocal(rinv[:, :npar], d3[:, :npar])
            wb = wp.tile([128, 512], dt.bfloat16, name="wb", tag="wb")
            wb3 = wb[:, :ncn].rearrange("p (n k) -> p n k", k=3)
            for kk in range(3):
                nc.vector.tensor_mul(wb3[:, :, kk], eb3[:, :, kk], rinv[:, :npar])
            # weighted child sum; alternate engines by e so hs8 pairs land
            # early for the next level's DoubleRow gh matmuls
            for e in range(EC):
                eng = nc.vector if e % 2 == 0 else nc.gpsimd
                pp = wp.tile([128, 512], dt.bfloat16, name="pp", tag="pp")
                eng.tensor_mul(pp[:, :ncn], hch[:, e, :ncn], wb[:, :ncn])
                p3 = pp[:, :ncn].rearrange("p (n k) -> p n k", k=3)
                ta = wp.tile([128, 176], dt.bfloat16, name="ta", tag="ta")
                eng.tensor_add(ta[:, :npar], p3[:, :, 0], p3[:, :, 1])
                eng.tensor_add(hs_sb[lp][:, e, p0:p0 + npar],
                               ta[:, :npar], p3[:, :, 2])
                if e % 2 == 1:
                    nc.vector.tensor_scalar(
                        hs8_sb[lp][:, e - 1:e + 1, p0:p0 + npar],
                        hs_sb[lp][:, e - 1:e + 1, p0:p0 + npar],
                        S_H, 0.0, Alu.mult, Alu.add)

        # first chunk's gathers go out before the bulk weight streams so the
        # leaf level can start immediately
        gxrows_cache = {0: emit_gather(SCHEDULE[0][3])}
        for g in range(EC):
            nc.sync.dma_start(out=ws[:, g, :, :, :],
                              in_=ws_d[g].rearrange("j p i m -> p j i m"))
        gorder = [base + e for e in range(EC) for base in (0, EC, 2 * EC)]
        for g in gorder:
            nc.sync.dma_start(out=whT[:, g, :, :, :],
                              in_=whT_d[g].rearrange("j p i m -> p j i m"))

        # interleaved pipeline: S1(i) ; S2(i-1)
        pending = {}
        for ci in range(len(SCHEDULE)):
            if ci + 1 < len(SCHEDULE):
                gxrows_cache[ci + 1] = emit_gather(SCHEDULE[ci + 1][3])
            hch, hch8 = emit_s1(ci, gxrows_cache.pop(ci))
            if SCHEDULE[ci][0] > 0:
                pending[ci] = (hch, hch8)
            if ci - 1 in pending:
                h_prev, h8_prev = pending.pop(ci - 1)
                emit_s2(ci - 1, h_prev, h8_prev)
        for ci in sorted(pending):
            emit_s2(ci, *pending[ci])

        # ---------------- output: transpose msb -> [T, E], DMA out
        osb = mp_.tile([T, E], dt.float32)
        for e in range(EC):
            po = psp.tile([128, 512], dt.float32, name="po", tag="ghn", bufs=2)
            nc.tensor.transpose(out=po[:T, :128], in_=msb[:, e, :],
                                identity=identf[:, :])
            nc.vector.tensor_copy(out=osb[:, e * 128:(e + 1) * 128], in_=po[:T, :128])
        nc.sync.dma_start(out=out_d[:, :], in_=osb[:, :])

    nc.compile()
    return nc


def get_nc():
    if "nc" not in _NC_CACHE:
        _NC_CACHE["nc"] = build_nc()
    return _NC_CACHE["nc"]


# ---------------------------------------------------------------- host side
def _prep_shared(emb, gru_Wi, gru_Wh, gru_bi, gru_bh, sent_weight, sent_bias,
                 context_weight):
    f32 = np.float32
    emb = np.ascontiguousarray(np.asarray(emb, f32))
    Wi = np.ascontiguousarray(np.asarray(gru_Wi, f32))
    bi = np.asarray(gru_bi, f32)
    bh = np.asarray(gru_bh, f32)
    # GX table: 64 * (emb @ Wi.T + bias_fold); bias_fold = (bi+bh) for r/z,
    # bi for n (bh_n enters via the r* coupling on-device)
    bias_fold = np.concatenate([(bi + bh)[:2 * E], bi[2 * E:]])
    GX = ((emb @ Wi.T + bias_fold) * S_GX).astype(bf16)

    def gmajor8(wT, ncols):
        # [E, ncols*128] -> [ncols, EC//2, 128, 2, 128] fp8 (DoubleRow pairs)
        a = wT.reshape(EC // 2, 2, 128, ncols, 128).transpose(3, 0, 2, 1, 4)
        return np.ascontiguousarray(np.clip(a * S_W, -240, 240)).astype(fp8t)
    whT = gmajor8(np.ascontiguousarray(np.asarray(gru_Wh, f32).T), 3 * EC)
    ws = gmajor8(np.ascontiguousarray(np.asarray(sent_weight, f32)), EC)
    ctxw = np.ascontiguousarray(np.asarray(context_weight, f32)).astype(bf16).reshape(EC, 128, 1)
    sb = np.asarray(sent_bias, f32).reshape(E)
    bias = np.zeros((128, 16), f32)
    for e in range(EC):
        bias[:, e] = S_GX * bh[2 * E + e * 128:2 * E + (e + 1) * 128]
        bias[:, 8 + e] = sb[e * 128:(e + 1) * 128]
    return GX, whT, ws, ctxw, bias


def _core_tokens(tokens, core):
    """Build the [NB, 128] int32 gather-index blocks for one core."""
    tok = np.asarray(tokens)[core * T:(core + 1) * T].astype(np.int32)
    blocks = np.zeros((NB, 128), np.int32)
    for (lvl, c0, ncn, blist) in SCHEDULE:
        flat = tok[:, LEVEL_OFF[lvl]:LEVEL_OFF[lvl] + K**lvl].reshape(-1)
        for (gb, boff, rows) in blist:
            blocks[gb, :rows] = flat[c0 + boff:c0 + boff + rows]
    return blocks


def kernel(tokens, bs, emb, gru_Wi, gru_Wh, gru_bi, gru_bh,
           sent_weight, sent_bias, context_weight, _trace=False):
    from concourse import bass_utils
    bass_utils.upload_artifacts = lambda tmpdir: "local://" + tmpdir

    nc = get_nc()
    GX, whT, ws, ctxw, bias = _prep_shared(
        emb, gru_Wi, gru_Wh, gru_bi, gru_bh, sent_weight, sent_bias, context_weight)

    in_maps = []
    for c in range(NCORES):
        in_maps.append({
            "tok": _core_tokens(tokens, c),
            "gx": GX, "whT": whT, "ws": ws, "ctxw": ctxw,
            "bias": bias,
        })
    res = bass_utils.run_bass_kernel_spmd(
        nc, in_maps, core_ids=list(range(NCORES)), trace=_trace)
    out = np.concatenate([res.results[c]["out"] for c in range(NCORES)], axis=0)
    if _trace:
        kernel.last_exec_time_ns = res.exec_time_ns
        kernel.last_results = res
    return out.astype(np.float32)


# revision 22
# speedup vs baseline: 2.0866x; 2.0866x over previous
"""Trainium2 Bass kernel for nn_BatchTreeEncoder (batched tree-GRU encoder).

Strategy
--------
Pure data parallel over the batch: 256 trees -> 32 trees on each of the 8
NeuronCores, weights replicated.  Activations are kept in a transposed
[E, nodes] layout (E-chunks of 128 on partitions, nodes on the free dim).

Key ideas:
  - GX[v] = 64*(Wi @ emb[v] + bias_fold) is precomputed on the HOST into a
    [VOCAB, 3E] bf16 table; the kernel indirect-DMA gathers GX rows and
    injects them transposed into the gate PSUM banks via identity matmuls
    (start=True), onto which the recurrent matmuls (Wh fp8 x8 DoubleRow on
    h fp8 x8 -> PSUM x64) accumulate (start=False).
  - r/z share one [128, 2, n] PSUM tile and a single merged activation.
  - software pipelining: chunk i's GRU (S1) is emitted before chunk i-1's
    attention (S2); small levels are split in two chunks so the pipeline
    crosses level boundaries (parents of chunk a only need children of
    chunk a).
  - attention softmax normalization runs 128-partition-wide after an
    all-partition broadcast of the exp row (matmul with ones), avoiding
    single-lane row ops; the weighted child sum runs on the Pool engine.
  - running per-tree elementwise max folded in as each H chunk completes.
Output: PE-transpose of the [E, 32] max back to [32, E] and DMA out.
"""

import sys

for _p in ("/opt/trn_rl_repo",):
    if _p not in sys.path:
        sys.path.insert(0, _p)

import numpy as np
import ml_dtypes

bf16 = ml_dtypes.bfloat16
fp8t = ml_dtypes.float8_e4m3

# ---------------------------------------------------------------- constants
NCORES = 8
BS = 256
T = BS // NCORES          # trees per core
K = 3
DEPTH = 4
E = 1024
EC = E // 128             # 8 e-chunks
VOCAB = 20000
N_NODES = sum(K**l for l in range(DEPTH + 1))   # 121
LEVEL_OFF = [sum(K**i for i in range(l)) for l in range(DEPTH + 1)]  # [0,1,4,13,40]

S_W = 8.0                 # fp8 weight scale (Wh, Ws)
S_H = 8.0                 # fp8 hidden scale
S_GX = S_W * S_H          # 64: GX table scale == gh psum scale

# node-chunk sizes per level; multiples of 3^l (whole trees inside one chunk
# for the max) and of 3 for l>0 (whole sibling groups for the attention).
# Levels are split >=2 ways so S1(i+1) can overlap S2(i) across levels.
CHUNK_SIZES = {4: [486] * 5 + [162], 3: [432, 432], 2: [288],
               1: [96], 0: [32]}


def _schedule():
    """Static per-core schedule: list of (level, c0, nc, [(gblock, boff, rows)])."""
    sched = []
    gb = 0
    for l in range(DEPTH, -1, -1):
        c0 = 0
        for nc_ in CHUNK_SIZES[l]:
            blocks = []
            boff = 0
            while boff < nc_:
                rows = min(128, nc_ - boff)
                blocks.append((gb, boff, rows))
                gb += 1
                boff += rows
            sched.append((l, c0, nc_, blocks))
            c0 += nc_
    return sched, gb


SCHEDULE, NB = _schedule()

_NC_CACHE = {}


# ---------------------------------------------------------------- builder
def build_nc():
    import concourse.bacc as bacc
    import concourse.bass as bass
    import concourse.mybir as mybir
    import concourse.tile as tile
    from concourse.masks import make_identity

    dt = mybir.dt
    Act = mybir.ActivationFunctionType
    Alu = mybir.AluOpType
    X = mybir.AxisListType.X

    nc = bacc.Bacc("TRN2", target_bir_lowering=False, debug=False)

    tok_d = nc.dram_tensor("tok", [NB, 128], dt.int32, kind="ExternalInput")
    gx_d = nc.dram_tensor("gx", [VOCAB, 3 * E], dt.bfloat16, kind="ExternalInput")
    whT_d = nc.dram_tensor("whT", [3 * EC, EC // 2, 128, 2, 128], dt.float8e4, kind="ExternalInput")
    ws_d = nc.dram_tensor("ws", [EC, EC // 2, 128, 2, 128], dt.float8e4, kind="ExternalInput")
    ctx_d = nc.dram_tensor("ctxw", [EC, 128, 1], dt.bfloat16, kind="ExternalInput")
    bias_d = nc.dram_tensor("bias", [128, 16], dt.float32, kind="ExternalInput")
    out_d = nc.dram_tensor("out", [T, E], dt.float32, kind="ExternalOutput")

    from contextlib import ExitStack

    DR = mybir.MatmulPerfMode.DoubleRow

    with tile.TileContext(nc) as tc, ExitStack() as ctx:
        sing = ctx.enter_context(tc.tile_pool(name="sing", bufs=1))
        hsp = ctx.enter_context(tc.tile_pool(name="hsp", bufs=1))
        mp_ = ctx.enter_context(tc.tile_pool(name="mp", bufs=1))
        gxp = ctx.enter_context(tc.tile_pool(name="gxp", bufs=7))
        gp = ctx.enter_context(tc.tile_pool(name="gp", bufs=2))      # gate tiles
        hp = ctx.enter_context(tc.tile_pool(name="hp", bufs=2))      # H chunks
        up = ctx.enter_context(tc.tile_pool(name="up", bufs=2))      # U tiles
        rowp = ctx.enter_context(tc.tile_pool(name="rowp", bufs=2))  # softmax rows
        wp = ctx.enter_context(tc.tile_pool(name="wp", bufs=2))      # bcast weights etc
        psp = ctx.enter_context(tc.tile_pool(name="psp", bufs=1, space="PSUM"))

        # ---- persistent / constant tiles
        whT = sing.tile([128, 3 * EC, EC // 2, 2, 128], dt.float8e4)
        ws = sing.tile([128, EC, EC // 2, 2, 128], dt.float8e4)
        ctxw = sing.tile([128, EC, 1], dt.bfloat16)
        biases = sing.tile([128, 16], dt.float32)
        identb = sing.tile([128, 128], dt.bfloat16)
        identf = sing.tile([128, 128], dt.float32)
        ones = sing.tile([1, 128], dt.bfloat16)
        idx = sing.tile([128, NB], dt.int32)

        nc.sync.dma_start(out=idx[:], in_=tok_d.rearrange("b p -> p b"))
        nc.sync.dma_start(out=biases[:], in_=bias_d[:])
        nc.sync.dma_start(out=ctxw[:, :, 0], in_=ctx_d.rearrange("k p o -> p (k o)"))
        make_identity(nc, identb[:])
        make_identity(nc, identf[:])
        nc.vector.memset(ones[:], 1.0)

        # bias column helpers: cols 0..7 = 64*bh_n, 8..15 = sent_bias
        def bcol(c):
            return biases[:, c:c + 1]

        # running max, [128, EC, T] f32
        msb = mp_.tile([128, EC, T], dt.float32)
        red = mp_.tile([128, EC, T], dt.float32)
        nc.vector.memset(msb[:], -3.0e38)

        # per-level HS accumulation targets ([E, N_l] as [128, EC, N_l])
        hs_sb = {}
        hs8_sb = {}
        for l in range(DEPTH):
            n_l = T * K**l
            hs_sb[l] = hsp.tile([128, EC, n_l], dt.bfloat16, name=f"hs{l}")
            hs8_sb[l] = hsp.tile([128, EC, n_l], dt.float8e4, name=f"hs8{l}")

        def gh_mms(out_ap, g, lvl, c0, ncn, start):
            src8 = hs8_sb[lvl]
            for j in range(EC // 2):
                nc.tensor.matmul(
                    out=out_ap, lhsT=whT[:, g, j, :, :],
                    rhs=src8[:, 2 * j:2 * j + 2, c0:c0 + ncn],
                    start=(start and j == 0), stop=(j == EC // 2 - 1),
                    perf_mode=DR, skip_group_check=not start)

        def u_mms(out_ap, f, h8, ncn):
            for j in range(EC // 2):
                nc.tensor.matmul(
                    out=out_ap, lhsT=ws[:, f, j, :, :],
                    rhs=h8[:, 2 * j:2 * j + 2, :ncn],
                    start=(j == 0), stop=(j == EC // 2 - 1), perf_mode=DR)

        # gather GX rows for one chunk; returns list of (tile, boff, rows)
        def emit_gather(blocks):
            rowsl = []
            for (gb, boff, rows) in blocks:
                gxrow = gxp.tile([128, 3 * E], dt.bfloat16, name="gxrow")
                nc.gpsimd.indirect_dma_start(
                    out=gxrow[:rows, :],
                    out_offset=None,
                    in_=gx_d[:, :],
                    in_offset=bass.IndirectOffsetOnAxis(ap=idx[:rows, gb:gb + 1], axis=0),
                )
                rowsl.append((gxrow, boff, rows))
            return rowsl

        # inject gathered gx gate-columns transposed into a psum region via a
        # regular matmul (out = gxrow_slice^T @ I); start=True resets the
        # region so the gh matmuls can accumulate on top with start=False
        def gx_tr(ps_ap_base, gxrows, gate, e, accum=False):
            col0 = gate * E + e * 128
            for (gxrow, boff, rows) in gxrows:
                nc.tensor.matmul(
                    out=ps_ap_base[:, boff:boff + rows],
                    lhsT=gxrow[:rows, col0:col0 + 128],
                    rhs=identb[:rows, :rows],
                    start=not accum, stop=True,
                    skip_group_check=accum,
                )

        # ---------------- S1: gather -> GRU gates -> H, hch8, running max
        def emit_s1(ci, gxrows):
            lvl, c0, ncn, blocks = SCHEDULE[ci]
            leaf = lvl == DEPTH
            n_per_tree = K**lvl
            tr0 = c0 // n_per_tree
            ntr = ncn // n_per_tree

            rz = gp.tile([128, 2, EC, 512], dt.bfloat16, name="rz", tag="rz")
            nt = gp.tile([128, EC, 512], dt.bfloat16, name="nt", tag="nt")
            hch = hp.tile([128, EC, 512], dt.bfloat16, name="hch")
            hch8 = None
            if lvl > 0:
                hch8 = hp.tile([128, EC, 512], dt.float8e4, name="hch8", tag="hch8")

            # r+z gates share one 2-bank psum and one merged activation
            for e in range(EC):
                psrz = psp.tile([128, 2, 512], dt.float32, name="psrz", tag="acc", bufs=2)
                gx_tr(psrz[:, 0, :], gxrows, 0, e)
                gx_tr(psrz[:, 1, :], gxrows, 1, e)
                if not leaf:
                    gh_mms(psrz[:, 0, :ncn], e, lvl, c0, ncn, start=False)
                    gh_mms(psrz[:, 1, :ncn], EC + e, lvl, c0, ncn, start=False)
                nc.scalar.activation(rz[:, :, e, :ncn], psrz[:, :, :ncn], Act.Tanh,
                                     scale=0.5 / S_GX)
            # r = 0.5 + 0.5*tau (batched over e)
            nc.vector.tensor_scalar(rz[:, 0, :, :ncn], rz[:, 0, :, :ncn], 0.5, 0.5,
                                    Alu.mult, Alu.add)
            # n gate
            for e in range(EC):
                psx = psp.tile([128, 512], dt.float32, name="psx", tag="gxn", bufs=2)
                gx_tr(psx, gxrows, 2, e)
                tt = gp.tile([128, 512], dt.float32, name="tt", tag="tt")
                if leaf:
                    # tt = r * 64bh_n + GXn64
                    nc.vector.scalar_tensor_tensor(
                        out=tt[:, :ncn], in0=rz[:, 0, e, :ncn], scalar=bcol(e),
                        in1=psx[:, :ncn], op0=Alu.mult, op1=Alu.add)
                else:
                    psh = psp.tile([128, 512], dt.float32, name="psh", tag="ghn", bufs=2)
                    gh_mms(psh[:, :ncn], 2 * EC + e, lvl, c0, ncn, start=True)
                    # tt = (GHn64 + 64bh_n) * r ; then += GXn64
                    nc.vector.scalar_tensor_tensor(
                        out=tt[:, :ncn], in0=psh[:, :ncn], scalar=bcol(e),
                        in1=rz[:, 0, e, :ncn], op0=Alu.add, op1=Alu.mult)
                    nc.vector.tensor_add(tt[:, :ncn], tt[:, :ncn], psx[:, :ncn])
                nc.scalar.activation(nt[:, e, :ncn], tt[:, :ncn], Act.Tanh,
                                     scale=1.0 / S_GX)
            # ---------------- blend -> H (batched over e, vector)
            zt = rz[:, 1, :, :ncn]
            nts = nt[:, :, :ncn]
            if leaf:
                nc.vector.tensor_scalar(zt, zt, -0.5, 0.5, Alu.mult, Alu.add)
                nc.vector.tensor_mul(hch[:, :, :ncn], zt, nts)
            else:
                dd = rz[:, 0, :, :ncn]      # r is dead; reuse as blend scratch
                nc.vector.tensor_sub(dd, hs_sb[lvl][:, :, c0:c0 + ncn], nts)
                nc.vector.tensor_mul(zt, zt, dd)
                nc.vector.tensor_add(dd, dd, zt)
                nc.vector.scalar_tensor_tensor(
                    out=hch[:, :, :ncn], in0=dd, scalar=0.5,
                    in1=nts, op0=Alu.mult, op1=Alu.add)
            if hch8 is not None:
                nc.vector.tensor_scalar(hch8[:, :, :ncn], hch[:, :, :ncn],
                                        S_H, 0.0, Alu.mult, Alu.add)
            # ---------------- running max (batched over e)
            if n_per_tree == 1:
                nc.vector.tensor_max(msb[:, :, tr0:tr0 + ntr],
                                     msb[:, :, tr0:tr0 + ntr], hch[:, :, :ncn])
            else:
                nc.vector.reduce_max(
                    out=red[:, :, :ntr],
                    in_=hch[:, :, :ncn].rearrange("p e (t n) -> p e t n",
                                                  n=n_per_tree),
                    axis=X)
                nc.vector.tensor_max(msb[:, :, tr0:tr0 + ntr],
                                     msb[:, :, tr0:tr0 + ntr], red[:, :, :ntr])
            return hch, hch8

        # ---------------- S2: child attention -> hs, hs8 for parent level
        def emit_s2(ci, hch, hch8):
            lvl, c0, ncn, blocks = SCHEDULE[ci]
            npar = ncn // 3
            p0 = c0 // 3
            lp = lvl - 1
            pss_t = psp.tile([128, 512], dt.float32, name="pss", tag="gxn", bufs=2)
            pss = pss_t[0:1, :]
            for f in range(EC):
                psu_t = psp.tile([128, 2, 512], dt.float32, name="psu", tag="acc", bufs=2)
                psu = psu_t[:, 0, :]
                u_mms(psu[:, :ncn], f, hch8, ncn)
                ut = up.tile([128, 512], dt.bfloat16, name="ut", tag="ut")
                nc.scalar.activation(ut[:, :ncn], psu[:, :ncn], Act.Tanh,
                                     bias=bcol(8 + f), scale=1.0 / S_GX)
                nc.tensor.matmul(out=pss[:, :ncn], lhsT=ctxw[:, f, 0:1],
                                 rhs=ut[:, :ncn],
                                 start=(f == 0), stop=(f == EC - 1))
            srow = rowp.tile([1, 512], dt.float32, name="srow", tag="srow")
            nc.scalar.activation(srow[:, :ncn], pss[:, :ncn], Act.Tanh)
            erow = rowp.tile([1, 512], dt.bfloat16, name="erow", tag="erow")
            nc.scalar.activation(erow[:, :ncn], srow[:, :ncn], Act.Exp)
            # broadcast exp row to 128 partitions, then normalize wide into
            # prenormalized weights wb (reading psw directly from PSUM)
            psw = psp.tile([128, 512], dt.float32, name="psw", tag="ghn", bufs=2)
            nc.tensor.matmul(out=psw[:, :ncn], lhsT=ones[:, :],
                             rhs=erow[:, :ncn], start=True, stop=True)
            eb = wp.tile([128, 512], dt.bfloat16, name="eb", tag="eb")
            nc.vector.tensor_copy(out=eb[:, :ncn], in_=psw[:, :ncn])
            eb3 = eb[:, :ncn].rearrange("p (n k) -> p n k", k=3)
            d3 = wp.tile([128, 176], dt.float32, name="d3", tag="d3")
            nc.vector.tensor_add(d3[:, :npar], eb3[:, :, 0], eb3[:, :, 1])
            nc.vector.tensor_add(d3[:, :npar], d3[:, :npar], eb3[:, :, 2])
            rinv = wp.tile([128, 176], dt.float32, name="rinv", tag="rinv")
            nc.vector.reciprocal(rinv[:, :npar], d3[:, :npar])
            wb = wp.tile([128, 512], dt.bfloat16, name="wb", tag="wb")
            wb3 = wb[:, :ncn].rearrange("p (n k) -> p n k", k=3)
            for kk in range(3):
                nc.vector.tensor_mul(wb3[:, :, kk], eb3[:, :, kk], rinv[:, :npar])
            # weighted child sum; hs8 pairs emitted as soon as both halves of
            # a DoubleRow pair are ready so the next level's gh can start
            for e in range(EC):
                pp = wp.tile([128, 512], dt.bfloat16, name="pp", tag="pp")
                nc.vector.tensor_mul(pp[:, :ncn], hch[:, e, :ncn], wb[:, :ncn])
                p3 = pp[:, :ncn].rearrange("p (n k) -> p n k", k=3)
                ta = wp.tile([128, 176], dt.bfloat16, name="ta", tag="ta")
                nc.vector.tensor_add(ta[:, :npar], p3[:, :, 0], p3[:, :, 1])
                nc.vector.tensor_add(hs_sb[lp][:, e, p0:p0 + npar],
                                     ta[:, :npar], p3[:, :, 2])
                if e % 2 == 1:
                    nc.vector.tensor_scalar(
                        hs8_sb[lp][:, e - 1:e + 1, p0:p0 + npar],
                        hs_sb[lp][:, e - 1:e + 1, p0:p0 + npar],
                        S_H, 0.0, Alu.mult, Alu.add)

        # first chunk's gathers go out before the bulk weight streams so the
        # leaf level can start immediately
        gxrows_cache = {0: emit_gather(SCHEDULE[0][3])}
        for g in range(EC):
            nc.sync.dma_start(out=ws[:, g, :, :, :],
                              in_=ws_d[g].rearrange("j p i m -> p j i m"))
        gorder = [base + e for e in range(EC) for base in (0, EC, 2 * EC)]
        for g in gorder:
            nc.sync.dma_start(out=whT[:, g, :, :, :],
                              in_=whT_d[g].rearrange("j p i m -> p j i m"))

        # interleaved pipeline: S1(i) ; S2(i-1) within a level.  At a level
        # boundary every pending S2 must be flushed first: the next level's
        # gh matmuls read the hs8 written by ALL of the previous level's S2s.
        pending = {}
        for ci in range(len(SCHEDULE)):
            if ci + 1 < len(SCHEDULE):
                gxrows_cache[ci + 1] = emit_gather(SCHEDULE[ci + 1][3])
            if ci > 0 and SCHEDULE[ci][0] != SCHEDULE[ci - 1][0]:
                for cj in sorted(pending):
                    emit_s2(cj, *pending.pop(cj))
            hch, hch8 = emit_s1(ci, gxrows_cache.pop(ci))
            if SCHEDULE[ci][0] > 0:
                pending[ci] = (hch, hch8)
            if ci - 1 in pending:
                h_prev, h8_prev = pending.pop(ci - 1)
                emit_s2(ci - 1, h_prev, h8_prev)
        for ci in sorted(pending):
            emit_s2(ci, *pending.pop(ci))

        # ---------------- output: transpose msb -> [T, E], DMA out
        osb = mp_.tile([T, E], dt.float32)
        for e in range(EC):
            po = psp.tile([128, 512], dt.float32, name="po", tag="ghn", bufs=2)
            nc.tensor.transpose(out=po[:T, :128], in_=msb[:, e, :],
                                identity=identf[:, :])
            nc.vector.tensor_copy(out=osb[:, e * 128:(e + 1) * 128], in_=po[:T, :128])
        nc.sync.dma_start(out=out_d[:, :], in_=osb[:, :])

    nc.compile()
    return nc


def get_nc():
    if "nc" not in _NC_CACHE:
        _NC_CACHE["nc"] = build_nc()
    return _NC_CACHE["nc"]


# ---------------------------------------------------------------- host side
def _prep_shared(emb, gru_Wi, gru_Wh, gru_bi, gru_bh, sent_weight, sent_bias,
                 context_weight):
    f32 = np.float32
    emb = np.ascontiguousarray(np.asarray(emb, f32))
    Wi = np.ascontiguousarray(np.asarray(gru_Wi, f32))
    bi = np.asarray(gru_bi, f32)
    bh = np.asarray(gru_bh, f32)
    # GX table: 64 * (emb @ Wi.T + bias_fold); bias_fold = (bi+bh) for r/z,
    # bi for n (bh_n enters via the r* coupling on-device)
    bias_fold = np.concatenate([(bi + bh)[:2 * E], bi[2 * E:]])
    GX = ((emb @ Wi.T + bias_fold) * S_GX).astype(bf16)

    def gmajor8(wT, ncols):
        # [E, ncols*128] -> [ncols, EC//2, 128, 2, 128] fp8 (DoubleRow pairs)
        a = wT.reshape(EC // 2, 2, 128, ncols, 128).transpose(3, 0, 2, 1, 4)
        return np.ascontiguousarray(np.clip(a * S_W, -240, 240)).astype(fp8t)
    whT = gmajor8(np.ascontiguousarray(np.asarray(gru_Wh, f32).T), 3 * EC)
    ws = gmajor8(np.ascontiguousarray(np.asarray(sent_weight, f32)), EC)
    ctxw = np.ascontiguousarray(np.asarray(context_weight, f32)).astype(bf16).reshape(EC, 128, 1)
    sb = np.asarray(sent_bias, f32).reshape(E)
    bias = np.zeros((128, 16), f32)
    for e in range(EC):
        bias[:, e] = S_GX * bh[2 * E + e * 128:2 * E + (e + 1) * 128]
        bias[:, 8 + e] = sb[e * 128:(e + 1) * 128]
    return GX, whT, ws, ctxw, bias


def _core_tokens(tokens, core):
    """Build the [NB, 128] int32 gather-index blocks for one core."""
    tok = np.asarray(tokens)[core * T:(core + 1) * T].astype(np.int32)
    blocks = np.zeros((NB, 128), np.int32)
    for (lvl, c0, ncn, blist) in SCHEDULE:
        flat = tok[:, LEVEL_OFF[lvl]:LEVEL_OFF[lvl] + K**lvl].reshape(-1)
        for (gb, boff, rows) in blist:
            blocks[gb, :rows] = flat[c0 + boff:c0 + boff + rows]
    return blocks


def kernel(tokens, bs, emb, gru_Wi, gru_Wh, gru_bi, gru_bh,
           sent_weight, sent_bias, context_weight, _trace=False):
    from concourse import bass_utils
    bass_utils.upload_artifacts = lambda tmpdir: "local://" + tmpdir

    nc = get_nc()
    GX, whT, ws, ctxw, bias = _prep_shared(
        emb, gru_Wi, gru_Wh, gru_bi, gru_bh, sent_weight, sent_bias, context_weight)

    in_maps = []
    for c in range(NCORES):
        in_maps.append({
            "tok": _core_tokens(tokens, c),
            "gx": GX, "whT": whT, "ws": ws, "ctxw": ctxw,
            "bias": bias,
        })
    res = bass_utils.run_bass_kernel_spmd(
        nc, in_maps, core_ids=list(range(NCORES)), trace=_trace)
    out = np.concatenate([res.results[c]["out"] for c in range(NCORES)], axis=0)
    if _trace:
        kernel.last_exec_time_ns = res.exec_time_ns
        kernel.last_results = res
    return out.astype(np.float32)
